# revision 14
# baseline (speedup 1.0000x reference)
"""Trainium2 Bass kernel for nn_EMHA (strided sparse attention block).

Math (per batch b of 4):
  XR = Wr @ x[b] + br                       (512, 4096)
  H  = raw view of XR as (4096, 512)        [free reshape in DRAM/flat space]
  q/k/v = per-64-col-block H @ W{q,k,v}.T   (same 64x64 W for all 8 head-blocks)
  The (B,N,M,HD)->(B,N/S,M,S,HD) raw reshape + einsums reduce exactly to:
  32 independent attention groups (r = n%4, m = head): rows n==r (mod 4),
  cols [64m,64m+64), each a (1024 x 1024) softmax attention.
  OutMat (4096,512) viewed as (512,4096); out[b] = We @ OutMat_view + be.

Sharding: 8 cores = (b in 0..4) x (head-group hg in 0..2, 4 heads each).
A core only needs x / produces out columns n' with (n'%512)//256 == hg
(8 interleaved 256-wide stripes), so there is no inter-core communication.

Per-core layout trick: computing H.T directly, positions stored in
"sigma order" sigma = g*512 + cc where n = 8*cc + g. In this order each
attention group's 1024 positions are two contiguous 512-blocks (g=r even i,
g=r+4 odd i), and the group's output rows land as contiguous blocks of
OutMat. All verified bit-close vs reference in numpy (mock.py).
"""

import numpy as np

EMBED, M, S, HD = 1024, 8, 4, 64
B, N = 4, 4096
NCORES = 8

_SCALE = 1.0 / 32.0  # 1/sqrt(EMBED)


def _build_nc(pack_e=True):
    import concourse.tile as tile
    from concourse import bacc, mybir

    dt = mybir.dt
    f32 = dt.float32
    f32r = dt.float32r
    bf16 = dt.bfloat16

    nc = bacc.Bacc(None, target_bir_lowering=False)

    xs = nc.dram_tensor("xs", [8, 1024, 256], f32r, kind="ExternalInput")
    wrt = nc.dram_tensor("wrt", [1024, 512], f32r, kind="ExternalInput")
    brb = nc.dram_tensor("brb", [128, 512], f32, kind="ExternalInput")
    bdq = nc.dram_tensor("bdq", [128, 128], f32r, kind="ExternalInput")
    bdk = nc.dram_tensor("bdk", [128, 128], f32r, kind="ExternalInput")
    bdv = nc.dram_tensor("bdv", [128, 128], f32r, kind="ExternalInput")
    wet = nc.dram_tensor("wet", [512, 1024], f32r, kind="ExternalInput")
    beb = nc.dram_tensor("beb", [128, 8], f32, kind="ExternalInput")
    out = nc.dram_tensor("out", [1024, 2048], f32, kind="ExternalOutput")

    with tile.TileContext(nc) as tc:
        with (
            tc.tile_pool(name="persist", bufs=1) as persist,
            tc.tile_pool(name="big", bufs=4) as bigpool,
            tc.tile_pool(name="wmat", bufs=1) as wmatp,
            tc.tile_pool(name="xin", bufs=2) as xin,
            tc.tile_pool(name="outp", bufs=3) as outp,
            tc.tile_pool(name="small", bufs=4) as small,
            tc.tile_pool(name="ps512", bufs=2, space="PSUM") as ps512,
            tc.tile_pool(name="pse", bufs=2, space="PSUM") as pse,
            tc.tile_pool(name="ps128", bufs=2, space="PSUM") as ps128,
        ):
            # ---- load weights/constants ----
            wrt_sb = wmatp.tile([128, 8, 512], f32r, tag="wmat")
            for kc in range(8):
                nc.sync.dma_start(wrt_sb[:, kc, :],
                                  wrt[kc * 128:(kc + 1) * 128, :])
            brb_sb = persist.tile([128, 512], f32, tag="brb")
            nc.sync.dma_start(brb_sb[:], brb[:])
            bdq_sb = persist.tile([128, 128], f32r, tag="bdq")
            nc.sync.dma_start(bdq_sb[:], bdq[:])
            bdk_sb = persist.tile([128, 128], f32r, tag="bdk")
            nc.sync.dma_start(bdk_sb[:], bdk[:])
            bdv_sb = persist.tile([128, 128], f32r, tag="bdv")
            nc.sync.dma_start(bdv_sb[:], bdv[:])
            beb_sb = persist.tile([128, 8], f32, tag="beb")
            nc.sync.dma_start(beb_sb[:], beb[:])

            # ---- stage 1: XRT matmuls -> HT (2 pair-tiles, fp32) ----
            # HT[p][cl, g, cc] = H.T[hg*256 + 128p + cl, 8cc+g]
            HT = [bigpool.tile([128, 8, 512], f32r, tag="big", name=f"HT{p}")
                  for p in range(2)]
            for g in range(8):
                x_sb = xin.tile([128, 8, 256], f32r, tag="xin")
                for kc in range(8):
                    nc.sync.dma_start(x_sb[:, kc, :],
                                      xs[g, kc * 128:(kc + 1) * 128, :])
                for p in range(2):
                    acc = ps512.tile([128, 512], f32, tag="ps512")
                    for kc in range(8):
                        nc.tensor.matmul(
                            acc[:],
                            x_sb[:, kc, p * 128:(p + 1) * 128],
                            wrt_sb[:, kc, :],
                            start=(kc == 0),
                            stop=(kc == 7),
                        )
                    nc.vector.tensor_add(HT[p][:, g, :], acc[:], brb_sb[:])

            # ---- stage 2: QKV ----
            qT = [persist.tile([128, 8, 512], bf16, tag=f"qT{p}", name=f"qT{p}")
                  for p in range(2)]
            kT = [persist.tile([128, 8, 512], bf16, tag=f"kT{p}", name=f"kT{p}")
                  for p in range(2)]
            # V_sb[p][sig, sb, grp*65 + c]; col 64 of each 65-block = ones
            V_sb = [persist.tile([128, 32, 130], bf16, tag=f"V{p}", name=f"V{p}")
                    for p in range(2)]
            for p in range(2):
                for ch in range(8):
                    pq = ps512.tile([128, 512], f32, tag="ps512")
                    nc.tensor.matmul(pq[:], bdq_sb[:], HT[p][:, ch, :],
                                     start=True, stop=True)
                    nc.vector.tensor_copy(out=qT[p][:, ch, :], in_=pq[:])
                    pk = ps512.tile([128, 512], f32, tag="ps512")
                    nc.tensor.matmul(pk[:], bdk_sb[:], HT[p][:, ch, :],
                                     start=True, stop=True)
                    nc.vector.tensor_copy(out=kT[p][:, ch, :], in_=pk[:])
                for sb in range(32):
                    pv = ps128.tile([128, 128], f32, tag="ps128")
                    nc.tensor.matmul(
                        pv[:],
                        HT[p][:, sb // 4, (sb % 4) * 128:(sb % 4) * 128 + 128],
                        bdv_sb[:],
                        start=True, stop=True,
                    )
                    nc.vector.tensor_copy(out=V_sb[p][:, sb, 0:64], in_=pv[:, 0:64])
                    nc.vector.tensor_copy(out=V_sb[p][:, sb, 65:129], in_=pv[:, 64:128])
                nc.vector.memset(V_sb[p][:, :, 64:65], 1.0)
                nc.vector.memset(V_sb[p][:, :, 129:130], 1.0)

            # ---- stage 3: attention rounds + stage 4 interleaved ----
            # round t: r = t//2, pair p = t%2; two groups (heads) per round
            OutMat = persist.tile([128, 4, 2048], f32r, tag="outmat")
            wet_sb = wmatp.tile([128, 4, 1024], f32r, tag="wmat")
            for cc in range(4):
                nc.sync.dma_start(wet_sb[:, cc, :],
                                  wet[cc * 128:(cc + 1) * 128, :])

            def emit_et(t):
                """E^T matmuls + exp for round t -> returns expE tiles [A, B]."""
                rr, p = t // 2, t % 2
                ee = [bigpool.tile([128, 8, 1024], bf16, tag="big",
                                   name=f"ee{t}_{g_}") for g_ in range(2)]
                for jb in range(8):
                    gj = rr if jb < 4 else rr + 4
                    cj = (jb % 4) * 128
                    pe_t = [pse.tile([128, 1024], f32, tag="pse",
                                     name=f"pe{t}_{jb}_{g_}") for g_ in range(2)]
                    for grp in range(2):
                        rows = slice(grp * 64, grp * 64 + 64)
                        for ic in range(2):
                            gi = rr if ic == 0 else rr + 4
                            kw = dict(start=True, stop=True)
                            if pack_e:
                                kw["tile_position"] = (grp * 64, 0)
                            nc.tensor.matmul(
                                pe_t[grp][:, ic * 512:(ic + 1) * 512],
                                kT[p][rows, gj, cj:cj + 128],
                                qT[p][rows, gi, :],
                                **kw,
                            )
                    for grp in range(2):
                        nc.scalar.activation(
                            out=ee[grp][:, jb, :],
                            in_=pe_t[grp][:],
                            func=mybir.ActivationFunctionType.Exp,
                            scale=_SCALE,
                        )
                return ee

            def emit_av(t, ee):
                rr, p = t // 2, t % 2
                for grp in range(2):
                    mloc = p * 2 + grp
                    for ib in range(8):
                        po = ps128.tile([128, 128], f32, tag="ps128")
                        for jc in range(8):
                            sbj = 4 * rr + jc if jc < 4 else 4 * (rr + 4) + (jc - 4)
                            nc.tensor.matmul(
                                po[:, 0:65],
                                ee[grp][:, jc, ib * 128:ib * 128 + 128],
                                V_sb[p][:, sbj, grp * 65:grp * 65 + 65],
                                start=(jc == 0),
                                stop=(jc == 7),
                            )
                        rec = small.tile([128, 1], f32, tag="rec")
                        nc.vector.reciprocal(out=rec[:], in_=po[:, 64:65])
                        u = rr if ib < 4 else rr + 4
                        col = u * 256 + mloc * 64
                        nc.vector.tensor_scalar_mul(
                            out=OutMat[:, ib % 4, col:col + 64],
                            in0=po[:, 0:64],
                            scalar1=rec[:],
                        )

            def emit_we(ncc):
                for ob in range(8):
                    pf = ps512.tile([128, 512], f32, tag="ps512")
                    for cc in range(4):
                        nc.tensor.matmul(
                            pf[:],
                            wet_sb[:, cc, ob * 128:ob * 128 + 128],
                            OutMat[:, cc, ncc * 512:(ncc + 1) * 512],
                            start=(cc == 0),
                            stop=(cc == 3),
                        )
                    ot = outp.tile([128, 512], f32, tag="outp")
                    nc.vector.tensor_scalar_add(
                        out=ot[:], in0=pf[:], scalar1=beb_sb[:, ob:ob + 1])
                    nc.sync.dma_start(
                        out[ob * 128:(ob + 1) * 128, ncc * 512:(ncc + 1) * 512], ot[:])

            prev = emit_et(0)
            for t in range(1, 8):
                cur = emit_et(t)
                emit_av(t - 1, prev)
                prev = cur
                if t == 4:
                    emit_we(0)  # rounds 0..3 (r=0,1) fill u in {0,1,4,5}
                    emit_we(2)
            emit_av(7, prev)
            emit_we(1)
            emit_we(3)

    nc.finalize()
    return nc


def _prep_inputs(x, Wq, Wk, Wv, Wr, br, We, be):
    x = np.ascontiguousarray(np.asarray(x, np.float32))
    wrt = np.ascontiguousarray(np.asarray(Wr, np.float32).T)
    wet = np.ascontiguousarray(np.asarray(We, np.float32).T)
    brb = np.ascontiguousarray(
        np.broadcast_to(np.asarray(br, np.float32)[None, :], (128, 512)))
    beb = np.ascontiguousarray(np.asarray(be, np.float32).reshape(8, 128).T)

    def bd(w):
        z = np.zeros((128, 128), np.float32)
        wt = np.asarray(w, np.float32).T
        z[:64, :64] = wt
        z[64:, 64:] = wt
        return z

    bdq, bdk, bdv = bd(Wq), bd(Wk), bd(Wv)
    shared = dict(wrt=wrt, wet=wet, brb=brb, beb=beb, bdq=bdq, bdk=bdk, bdv=bdv)
    in_maps = []
    for core in range(NCORES):
        b, hg = core // 2, core % 2
        xs = np.ascontiguousarray(
            x[b].reshape(1024, 8, 2, 256)[:, :, hg, :].transpose(1, 0, 2))
        in_maps.append(dict(xs=xs, **shared))
    return in_maps


def kernel(x, Wq, Wk, Wv, Wr, br, We, be, _trace=False, _pack_e=True):
    from concourse.bass_utils import run_bass_kernel_spmd

    nc = _build_nc(pack_e=_pack_e)
    in_maps = _prep_inputs(x, Wq, Wk, Wv, Wr, br, We, be)
    res = run_bass_kernel_spmd(nc, in_maps, core_ids=list(range(NCORES)),
                               trace=_trace)
    out = np.zeros((B, EMBED, N), np.float32)
    for core in range(NCORES):
        b, hg = core // 2, core % 2
        oc = res.results[core]["out"]
        out[b].reshape(1024, 8, 2, 256)[:, :, hg, :] = oc.reshape(1024, 8, 256)
    if _trace:
        kernel._last_results = res
    return out
